# revision 18
# baseline (speedup 1.0000x reference)
"""Trainium2 Bass kernel for a 127-step flow-matching sampler.

Model: x_{k+1} = x_k - (t-tprev)*net(t, x_k) + sigma_k*noise_k, where net is a
5-layer MLP (33 -> 512 -> 512 -> 512 -> 512 -> 16) applied to
concat([t*ones, x_t, x_1]) over a batch of 32768 rows.

Strategy (pure data parallel over 8 NeuronCores, 4096 rows each):
  - Activations live feature-on-partition: A = h^T with shape [feat, batch].
    Then h_next^T = relu(W^T @ h^T + b) maps onto the PE with lhsT = W
    (natural [in, out] layout) and rhs = A, accumulating K in PSUM.
  - The t*ones input column is folded into a per-step layer-0 bias
    (b0_eff = b0 + t*w0[0,:]) computed on host, so the device-side input is
    just [x_t; x_1] with K=32.
  - Per-step noise is input-independent (jax threefry of a fixed key), so it
    is precomputed on host (exact numpy replica of jax.random.normal),
    pre-scaled by sigma, with -c*b4 folded in, transposed, and streamed
    from HBM.
  - x_t state is updated in place in SBUF; only the final x is written out.
"""

import os
import numpy as np

import concourse.bass as bass
import concourse.tile as tile
from concourse import bacc, mybir
from concourse.bass_utils import run_bass_kernel_spmd

# ---------------------------------------------------------------------------
# Problem constants (hardcoded per contract)
# ---------------------------------------------------------------------------
N_CORES = 8
BATCH = 32768
B = BATCH // N_CORES          # 4096 rows per core
DX = 16
HID = 512
N_STEPS = 128                 # linspace points; 127 integration steps
NOISE_SCALE = 0.01
CH = 512                      # batch columns per chunk (matmul moving dim)
NCH = B // CH                 # 8 chunks per core

_MM_DT_NAME = os.environ.get("KERNEL_MM_DT", "float32r")
_N_STEPS_OVR = int(os.environ.get("KERNEL_N_STEPS", str(N_STEPS)))
_UNROLL = os.environ.get("KERNEL_UNROLL", "0") == "1"


# ---------------------------------------------------------------------------
# Per-step noise: input-independent, generated with jax.random exactly as the
# reference does (scan + fold_in + normal), on the process-default backend so
# the PRNG impl/backend matches whatever the reference computation uses.
# ---------------------------------------------------------------------------
_NOISE_CACHE = {}


def host_noise(n_steps=N_STEPS):
    """Returns noise[i] for i in 0..n_steps-2, each [BATCH, DX] float32."""
    if n_steps in _NOISE_CACHE:
        return _NOISE_CACHE[n_steps]
    import jax
    import jax.numpy as jnp

    noise_key = jax.random.key(42)

    def body(carry, i):
        z = jax.random.normal(jax.random.fold_in(noise_key, i),
                              (BATCH, DX), jnp.float32)
        return carry, z

    @jax.jit
    def gen():
        _, zs = jax.lax.scan(body, 0, jnp.arange(n_steps - 1))
        return zs

    out = np.asarray(gen())
    _NOISE_CACHE[n_steps] = out
    return out


def host_schedule(n_steps=N_STEPS):
    """t, tprev, c=(t-tprev), sigma arrays as float32 (matching jax linspace)."""
    steps = np.linspace(1.0, 0.0, n_steps).astype(np.float32)
    t = steps[:-1]
    tp = steps[1:]
    c = (t - tp).astype(np.float32)
    sigma = np.sqrt(NOISE_SCALE * tp * c / t).astype(np.float32)
    return t, tp, c, sigma


# ---------------------------------------------------------------------------
# Device kernel
# ---------------------------------------------------------------------------
def build_nc(n_iters, mm_dt_name="float32r", unroll=False):
    """Build + compile the Bass program for one core (SPMD across 8)."""
    mm_dt = getattr(mybir.dt, mm_dt_name)
    f32 = mybir.dt.float32
    is_f32r = mm_dt == mybir.dt.float32r

    nc = bacc.Bacc("TRN2", target_bir_lowering=False, debug=False,
                   num_devices=N_CORES)

    # DRAM I/O (per-core shapes); all inputs arrive as f32, cast on device
    x1Tf = nc.dram_tensor("x1Tf", [DX, B], f32, kind="ExternalInput").ap()
    w0p = nc.dram_tensor("w0p", [32, HID], f32, kind="ExternalInput").ap()
    w1 = nc.dram_tensor("w1", [HID, HID], f32, kind="ExternalInput").ap()
    w2 = nc.dram_tensor("w2", [HID, HID], f32, kind="ExternalInput").ap()
    w3 = nc.dram_tensor("w3", [HID, HID], f32, kind="ExternalInput").ap()
    w4 = nc.dram_tensor("w4", [HID, DX], f32, kind="ExternalInput").ap()
    b0e = nc.dram_tensor("b0e", [n_iters, HID], f32, kind="ExternalInput").ap()
    b1 = nc.dram_tensor("b1", [HID], f32, kind="ExternalInput").ap()
    b2 = nc.dram_tensor("b2", [HID], f32, kind="ExternalInput").ap()
    b3 = nc.dram_tensor("b3", [HID], f32, kind="ExternalInput").ap()
    cneg = nc.dram_tensor("cneg", [n_iters, DX], f32, kind="ExternalInput").ap()
    noise = nc.dram_tensor("noise", [n_iters, DX, B], f32, kind="ExternalInput").ap()
    out = nc.dram_tensor("out", [DX, B], f32, kind="ExternalOutput").ap()

    io = locals()
    with tile.TileContext(nc) as tc:
        from contextlib import ExitStack
        with ExitStack() as ctx:
            _emit(ctx, tc, n_iters, mm_dt, io, unroll)
    nc.compile()
    return nc


def _emit(ctx, tc, n_iters, mm_dt, io, unroll):
    nc = tc.nc
    f32 = mybir.dt.float32
    is_f32r = mm_dt == mybir.dt.float32r
    Relu = mybir.ActivationFunctionType.Relu
    Alu = mybir.AluOpType
    ds = bass.ds

    w0p, w1, w2, w3, w4 = (io[k] for k in ("w0p", "w1", "w2", "w3", "w4"))
    b0e, b1, b2, b3, cneg, noise, out = (
        io[k] for k in ("b0e", "b1", "b2", "b3", "cneg", "noise", "out"))
    x1Tf = io["x1Tf"]

    singles = ctx.enter_context(tc.tile_pool(name="singles", bufs=1))
    acts = ctx.enter_context(tc.tile_pool(name="acts", bufs=10))
    psum = ctx.enter_context(tc.tile_pool(name="psum", bufs=8, space="PSUM"))
    small = ctx.enter_context(tc.tile_pool(name="small", bufs=4))
    upd = ctx.enter_context(tc.tile_pool(name="upd", bufs=6))
    scratch = ctx.enter_context(tc.tile_pool(name="scratch", bufs=2))

    # ---- persistent weights (DMA as f32, cast to mm_dt on device) -----------
    def load_cast(dst, src_ap):
        s = scratch.tile(list(dst.shape), f32, tag="wload")
        nc.sync.dma_start(out=s, in_=src_ap)
        nc.vector.tensor_copy(out=dst, in_=s)

    w0t = singles.tile([32, HID], mm_dt, tag="w0t")
    load_cast(w0t, w0p)
    wts = []
    for name, w in (("w1", w1), ("w2", w2), ("w3", w3)):
        wt = singles.tile([128, 4, HID], mm_dt, tag=f"{name}t")
        load_cast(wt, w.rearrange("(k p) j -> p k j", p=128))
        wts.append(wt)
    w4t = singles.tile([128, 4, DX], mm_dt, tag="w4t")
    load_cast(w4t, w4.rearrange("(k p) j -> p k j", p=128))
    bts = []
    for name, b in (("b1", b1), ("b2", b2), ("b3", b3)):
        bt = singles.tile([128, 4], f32, tag=f"{name}t")
        nc.sync.dma_start(out=bt, in_=b.rearrange("(m p) -> p m", p=128))
        bts.append(bt)
    b1t, b2t, b3t = bts

    # ---- persistent state: x_t (f32) and A0 = [x_t; x_1] in mm_dt -----------
    a0 = []
    xt = []
    for n in range(NCH):
        cols = slice(n * CH, (n + 1) * CH)
        a0c = singles.tile([32, CH], mm_dt, tag=f"a0_{n}")
        xtc = singles.tile([DX, CH], f32, tag=f"xt_{n}")
        nc.sync.dma_start(out=xtc, in_=x1Tf[:, cols])
        s = scratch.tile([32, CH], f32, tag="a0init")
        nc.sync.dma_start(out=s[0:DX, :], in_=x1Tf[:, cols])
        nc.sync.dma_start(out=s[DX:32, :], in_=x1Tf[:, cols])
        nc.vector.tensor_copy(out=a0c, in_=s)
        a0.append(a0c)
        xt.append(xtc)

    def xt_f32(n):
        return xt[n]

    # ---- per-step constants, preloaded to SBUF ------------------------------
    # b0all[p, s, m] = b0_eff[s, m*128+p];  cnall[d, s] = -c_s
    b0all = singles.tile([128, n_iters, 4], f32, tag="b0all")
    nc.sync.dma_start(out=b0all, in_=b0e.rearrange("s (m p) -> p s m", p=128))
    cnall = singles.tile([DX, n_iters], f32, tag="cnall")
    nc.sync.dma_start(out=cnall, in_=cneg.rearrange("s d -> d s"))

    def step_body(i, dyn):
        if dyn:
            # Register-offset APs are only reliable on plain copies: stage the
            # per-step constants into small tiles, then slice statically.
            b0t = small.tile([128, 4], f32, tag="b0t")
            nc.vector.tensor_copy(
                out=b0t, in_=b0all[:, ds(i, 1), :].rearrange("p o m -> p (o m)"))
            cnt_t = small.tile([DX, 1], f32, tag="cnt")
            nc.vector.tensor_copy(out=cnt_t, in_=cnall[:, ds(i, 1)])
            b0_bias = lambda m: b0t[:, m:m + 1]
            cnt = cnt_t[:, 0:1]
            nz_src = lambda cols: noise[ds(i, 1), :, cols].rearrange(
                "o d c -> (o d) c")
        else:
            b0_bias = lambda m: b0all[:, i, m:m + 1]
            cnt = cnall[:, i:i + 1]
            nz_src = lambda cols: noise[i, :, cols]

        for n in range(NCH):
            cols = slice(n * CH, (n + 1) * CH)
            nz = upd.tile([DX, CH], f32, tag="nz")
            nc.sync.dma_start(out=nz, in_=nz_src(cols))

            # L0: [32 -> 512]
            h = acts.tile([128, 4, CH], mm_dt, tag="h")
            for m in range(4):
                ps = psum.tile([128, CH], f32, tag="ps")
                nc.tensor.matmul(ps, w0t[:, m * 128:(m + 1) * 128], a0[n],
                                 start=True, stop=True)
                nc.scalar.activation(out=h[:, m, :], in_=ps, func=Relu,
                                     bias=b0_bias(m), scale=1.0)

            # L1..L3: [512 -> 512]
            for li, (wt, bt) in enumerate(((wts[0], b1t), (wts[1], b2t),
                                           (wts[2], b3t))):
                hn = acts.tile([128, 4, CH], mm_dt, tag="h")
                for m in range(4):
                    ps = psum.tile([128, CH], f32, tag="ps")
                    for k in range(4):
                        nc.tensor.matmul(ps, wt[:, k, m * 128:(m + 1) * 128],
                                         h[:, k, :], start=(k == 0), stop=(k == 3))
                    if li == 1:
                        nc.vector.tensor_scalar(
                            out=hn[:, m, :], in0=ps, scalar1=bt[:, m:m + 1],
                            scalar2=0.0, op0=Alu.add, op1=Alu.max)
                    else:
                        nc.scalar.activation(out=hn[:, m, :], in_=ps, func=Relu,
                                             bias=bt[:, m:m + 1], scale=1.0)
                h = hn

            # L4: [512 -> 16] (bias b4 folded into noise on host)
            ps4 = psum.tile([128, CH], f32, tag="ps")
            for k in range(4):
                nc.tensor.matmul(ps4[:DX, :], w4t[:, k, :], h[:, k, :],
                                 start=(k == 0), stop=(k == 3))

            # update: x_new = (drift_psum * -c) + x_t + sigma*noise - c*b4
            u = upd.tile([DX, CH], f32, tag="u")
            nc.vector.scalar_tensor_tensor(
                out=u, in0=ps4[:DX, :], scalar=cnt, in1=xt[n],
                op0=Alu.mult, op1=Alu.add)
            nc.vector.tensor_tensor(out=xt[n], in0=u, in1=nz, op=Alu.add)
            nc.vector.tensor_copy(out=a0[n][0:DX, :], in_=xt[n])

    # ---- step loop ----------------------------------------------------------
    if unroll:
        for i in range(n_iters):
            step_body(i, dyn=False)
    else:
        with tc.For_i(0, n_iters, 1,
                      hint_engines=(mybir.EngineType.PE,)) as iv:
            step_body(iv, dyn=True)

    # ---- writeback ----------------------------------------------------------
    for n in range(NCH):
        cols = slice(n * CH, (n + 1) * CH)
        nc.sync.dma_start(out=out[:, cols], in_=xt_f32(n))


# ---------------------------------------------------------------------------
# Host wrapper
# ---------------------------------------------------------------------------
_CACHE = {}


def _get_nc(n_iters, mm_dt_name, unroll=_UNROLL):
    key = (n_iters, mm_dt_name, unroll)
    if key not in _CACHE:
        _CACHE[key] = build_nc(n_iters, mm_dt_name, unroll)
    return _CACHE[key]


def prepare_in_maps(x_1, w0, b0, w1, b1, w2, b2, w3, b3, w4, b4,
                    n_steps=None, mm_dt_name=None):
    n_steps = n_steps or _N_STEPS_OVR
    mm_dt_name = mm_dt_name or _MM_DT_NAME
    n_iters = n_steps - 1

    t, tp, c, sigma = host_schedule(n_steps)
    b0e = (b0[None, :] + t[:, None] * w0[0:1, :]).astype(np.float32)
    cneg_arr = np.repeat((-c)[:, None], DX, axis=1).astype(np.float32)

    z = host_noise(n_steps)                      # [S, BATCH, DX]
    # noise_eff[i] = sigma_i * z_i^T - c_i * b4[:, None]   -> [S, DX, BATCH]
    zT = np.ascontiguousarray(z.transpose(0, 2, 1))
    noise_eff = (sigma[:, None, None] * zT
                 - c[:, None, None] * b4[None, :, None]).astype(np.float32)

    x1T_full = np.ascontiguousarray(x_1.astype(np.float32).T)  # [DX, BATCH]

    w0p_arr = np.ascontiguousarray(w0[1:33]).astype(np.float32)
    w_arrs = [w.astype(np.float32) for w in (w1, w2, w3, w4)]

    in_maps = []
    for cix in range(N_CORES):
        cols = slice(cix * B, (cix + 1) * B)
        m = {
            "x1Tf": np.ascontiguousarray(x1T_full[:, cols]),
            "w0p": w0p_arr,
            "w1": w_arrs[0], "w2": w_arrs[1], "w3": w_arrs[2], "w4": w_arrs[3],
            "b0e": b0e, "b1": b1.astype(np.float32), "b2": b2.astype(np.float32),
            "b3": b3.astype(np.float32),
            "cneg": cneg_arr,
            "noise": np.ascontiguousarray(noise_eff[:, :, cols]),
        }
        in_maps.append(m)
    return in_maps, (n_iters, mm_dt_name)


def run(in_maps, key, **kw):
    nc = _get_nc(*key)
    return run_bass_kernel_spmd(nc, in_maps, core_ids=list(range(N_CORES)), **kw)


def kernel(x_1, w0, b0, w1, b1, w2, b2, w3, b3, w4, b4):
    in_maps, key = prepare_in_maps(x_1, w0, b0, w1, b1, w2, b2, w3, b3, w4, b4)
    res = run(in_maps, key)
    parts = [res.results[c]["out"].T for c in range(N_CORES)]   # [B, DX] each
    return np.ascontiguousarray(np.concatenate(parts, axis=0)).astype(np.float32)
